# revision 18
# baseline (speedup 1.0000x reference)
"""Continuous exponential Koopman operator on 8 TRN2 NeuronCores.

Reference computes K = expm(kernel*dt) and the sequential scan
z_{t+1} = z_t @ K for T=1024 steps, returning all states [B, T, d].

Strategy (data-parallel over batch, 8 cores x 128 rows):
  - Host: expm + powers K^1..K^S in float64 (tiny: d=256), cast to fp32.
  - Device: the scan is re-associated into T/S blocks of S=32 steps.
    Within a block all S outputs depend only on the block-start state Z_b:
       out[:, b*S+j] = Z_b @ K^(j+1)   (one fat fp32r matmul sweep)
    and the sequential dependency is only the block chain
       Z_{b+1} = Z_b @ K^S             (exact-fp32 matmul)
    Z is kept transposed in SBUF ([d, B_local]) so both mappings need no
    on-device transposes:
      mapping A (outputs):  out[b,n]   = sum_k zT[k,b] * Kcat[k,n]  (lhsT=zT)
      mapping B (chain):    zT'[n,b]   = sum_k K^S[k,n] * zT[k,b]   (lhsT=K^S)
    fp32r runs the PE at full rate (1 cyc/row, ~12-bit mantissa); the
    chain stays exact fp32 so rounding never accumulates across blocks.
"""

import numpy as np

import concourse.mybir as mybir
from concourse import bacc
from concourse.bass_utils import run_bass_kernel_spmd
from concourse.tile import TileContext

F32 = mybir.dt.float32
F32R = mybir.dt.float32r

D = 256  # koopman dim
B = 1024  # batch
T_STEPS = 1024
DT = 0.01
N_CORES = 8
BL = B // N_CORES  # 128 batch rows per core
S = 32  # block size (timesteps per block)
NBLK = T_STEPS // S
FREE = 512  # matmul moving free dim (fp32 max)
CHUNKS = S * D // FREE  # 16 psum chunks per block

# set by test harness to request an NTFF profile; results land in _LAST_RESULT
_PROFILE = False
_LAST_RESULT = None
_NC_CACHE = None
_RUNNER = None


def _expm64(a: np.ndarray) -> np.ndarray:
    """Matrix exponential in float64 (scipy if present, else Pade 13)."""
    try:
        from scipy.linalg import expm

        return expm(a)
    except Exception:
        pass
    # Higham scaling-and-squaring with Pade 13
    b = (
        64764752532480000.0, 32382376266240000.0, 7771770303897600.0,
        1187353796428800.0, 129060195264000.0, 10559470521600.0,
        670442572800.0, 33522128640.0, 1323241920.0, 40840800.0,
        960960.0, 16380.0, 182.0, 1.0,
    )
    n = a.shape[0]
    nrm = np.linalg.norm(a, 1)
    s = max(0, int(np.ceil(np.log2(max(nrm / 5.371920351148152, 1e-300)))))
    a = a / (2.0**s)
    ident = np.eye(n)
    a2 = a @ a
    a4 = a2 @ a2
    a6 = a2 @ a4
    u = a @ (
        a6 @ (b[13] * a6 + b[11] * a4 + b[9] * a2)
        + b[7] * a6 + b[5] * a4 + b[3] * a2 + b[1] * ident
    )
    v = (
        a6 @ (b[12] * a6 + b[10] * a4 + b[8] * a2)
        + b[6] * a6 + b[4] * a4 + b[2] * a2 + b[0] * ident
    )
    r = np.linalg.solve(v - u, v + u)
    for _ in range(s):
        r = r @ r
    return r


def _build(repeat: int = 0, mode: str = "full"):
    """Per-core Tile program (identical on all 8 cores).

    repeat=0: production build — full [BL, T*D] ExternalOutput.
    repeat>=1: timing build — same work in a hardware For_i loop run
    `repeat` times against an Internal DRAM buffer (same DMA traffic),
    with only a tiny token ExternalOutput so wall-clock deltas between
    repeat counts measure pure device time.

    mode (timing ablations): "full" | "nodma" (no HBM output writes) |
    "dmaonly" (only the 32 output DMAs from one SBUF tile) |
    "nocopy" (matmuls only, no PSUM drains, no DMA).
    """
    nc = bacc.Bacc("TRN2", target_bir_lowering=False, debug=False,
                   num_devices=N_CORES)

    zt_d = nc.dram_tensor("zt", [D, BL], F32, kind="ExternalInput")
    kcat_d = nc.dram_tensor("kcat", [D, S * D], F32, kind="ExternalInput")
    ks_d = nc.dram_tensor("ks", [D, D], F32, kind="ExternalInput")
    if repeat:
        out_d = nc.dram_tensor("outbuf", [BL, T_STEPS * D], F32)
        tok_d = nc.dram_tensor("tok", [BL, FREE], F32, kind="ExternalOutput")
        if mode == "dmacontig":
            outc_d = nc.dram_tensor("outc", [NBLK * BL, S * D], F32)
    else:
        out_d = nc.dram_tensor("out", [BL, T_STEPS * D], F32,
                               kind="ExternalOutput")

    with TileContext(nc) as tc:
        with (
            tc.tile_pool(name="const", bufs=1) as cpool,
            tc.tile_pool(name="zp", bufs=2) as zpool,
            tc.tile_pool(name="obp", bufs=3 if mode.endswith("b3") else 2) as obpool,
            tc.tile_pool(name="po", bufs=6, space="PSUM") as popool,
            tc.tile_pool(name="pc", bufs=1, space="PSUM") as pcpool,
        ):
            # K powers, loaded fp32 (fast HWDGE) then rounded to fp32r on DVE
            # (gpsimd cast-DMA measured ~15 GB/s — 20x slower than this path)
            kcr0 = cpool.tile([128, S * D], F32R, name="kcr0")
            kcr1 = cpool.tile([128, S * D], F32R, name="kcr1")
            with tc.tile_pool(name="stage", bufs=2) as spool:
                qw = S * D // 4
                for q in range(4):
                    cols = slice(q * qw, (q + 1) * qw)
                    for kcr, rows in ((kcr0, slice(0, 128)),
                                      (kcr1, slice(128, 256))):
                        kst = spool.tile([128, qw], F32, name="kst")
                        nc.sync.dma_start(out=kst, in_=kcat_d[rows, cols])
                        nc.vector.tensor_copy(out=kcr[:, cols], in_=kst)
            # K^S for the exact-fp32 chain
            ks0 = cpool.tile([128, D], F32, name="ks0")
            ks1 = cpool.tile([128, D], F32, name="ks1")
            nc.sync.dma_start(out=ks0, in_=ks_d[0:128, :])
            nc.sync.dma_start(out=ks1, in_=ks_d[128:256, :])

            if mode.startswith("dmaonly") or mode == "dmacontig":
                obc = cpool.tile([128, S * D], F32, name="obc")
                nc.vector.memset(obc, 1.0)

            def body():
                if mode == "dmacontig":
                    # contiguous 4 MiB slab per DMA (partition stride 32 KiB)
                    for b in range(NBLK):
                        nc.sync.dma_start(
                            out=outc_d[b * BL : (b + 1) * BL, :], in_=obc
                        )
                    return
                if mode.startswith("dmaonly"):
                    # dmaonly: all on sync ring; dmaonly2: sync+scalar rings;
                    # dmaonly3: sync+scalar+gpsimd
                    eng = {
                        "dmaonly": [nc.sync],
                        "dmaonly2": [nc.sync, nc.scalar],
                        "dmaonly3": [nc.sync, nc.scalar, nc.gpsimd],
                    }[mode]
                    for b in range(NBLK):
                        eng[b % len(eng)].dma_start(
                            out=out_d[:, b * S * D : (b + 1) * S * D], in_=obc
                        )
                    return
                if mode == "dmaread":
                    for b in range(NBLK):
                        rb = obpool.tile([128, S * D], F32, name="ob")
                        nc.sync.dma_start(
                            out=rb, in_=out_d[:, b * S * D : (b + 1) * S * D]
                        )
                    return
                # block-start state, transposed [d, BL]; fp32 + fp32r copy
                zt0 = zpool.tile([128, BL], F32, name="zt0")
                zt1 = zpool.tile([128, BL], F32, name="zt1")
                nc.sync.dma_start(out=zt0, in_=zt_d[0:128, :])
                nc.sync.dma_start(out=zt1, in_=zt_d[128:256, :])
                ztr0 = zpool.tile([128, BL], F32R, name="ztr0")
                ztr1 = zpool.tile([128, BL], F32R, name="ztr1")
                nc.vector.tensor_copy(out=ztr0, in_=zt0)
                nc.vector.tensor_copy(out=ztr1, in_=zt1)

                for b in range(NBLK):
                    # chain first: PE computes Z_{b+1} while streaming block
                    # b's outputs, so the recurrence never stalls the PE
                    if b < NBLK - 1:
                        psc0 = pcpool.tile([128, BL], F32, name="psc0")
                        psc1 = pcpool.tile([128, BL], F32, name="psc1")
                        nc.tensor.matmul(psc0, ks0[:, 0:128], zt0,
                                         start=True, stop=False)
                        nc.tensor.matmul(psc0, ks1[:, 0:128], zt1,
                                         start=False, stop=True)
                        nc.tensor.matmul(psc1, ks0[:, 128:256], zt0,
                                         start=True, stop=False)
                        nc.tensor.matmul(psc1, ks1[:, 128:256], zt1,
                                         start=False, stop=True)
                        zt0n = zpool.tile([128, BL], F32, name="zt0")
                        zt1n = zpool.tile([128, BL], F32, name="zt1")
                        ztr0n = zpool.tile([128, BL], F32R, name="ztr0")
                        ztr1n = zpool.tile([128, BL], F32R, name="ztr1")
                        nc.scalar.copy(out=zt0n, in_=psc0)
                        nc.scalar.copy(out=zt1n, in_=psc1)
                        nc.vector.tensor_copy(out=ztr0n, in_=psc0)
                        nc.vector.tensor_copy(out=ztr1n, in_=psc1)

                    # block outputs: [BL, S*D] in 512-wide psum chunks
                    free = 256 if mode == "nocopy256" else FREE
                    nchunks = S * D // free
                    ob = obpool.tile([128, S * D], F32, name="ob")
                    for c in range(nchunks):
                        cols = slice(c * free, (c + 1) * free)
                        po = popool.tile([128, free], F32, name="po")
                        nc.tensor.matmul(po, ztr0, kcr0[:, cols],
                                         start=True, stop=False)
                        nc.tensor.matmul(po, ztr1, kcr1[:, cols],
                                         start=False, stop=True)
                        if not mode.startswith("nocopy"):
                            nc.vector.tensor_copy(out=ob[:, cols], in_=po)
                    if mode in ("full", "full2", "full2b3"):
                        # alternate the two HWDGE rings (SP / ACT) — measured
                        # ~25% faster than all-on-sync for this write stream
                        (nc.sync if b % 2 == 0 else nc.scalar).dma_start(
                            out=out_d[:, b * S * D : (b + 1) * S * D], in_=ob
                        )
                    elif mode == "fullsplit":
                        # both rings concurrently on each block (2 MiB halves)
                        h = S * D // 2
                        nc.sync.dma_start(
                            out=out_d[:, b * S * D : b * S * D + h],
                            in_=ob[:, 0:h],
                        )
                        nc.scalar.dma_start(
                            out=out_d[:, b * S * D + h : (b + 1) * S * D],
                            in_=ob[:, h:],
                        )

                    if b < NBLK - 1:
                        zt0, zt1, ztr0, ztr1 = zt0n, zt1n, ztr0n, ztr1n

            if repeat:
                with tc.For_i(0, repeat) as _i:
                    body()
                nc.sync.dma_start(out=tok_d[:, :],
                                  in_=out_d[:, T_STEPS * D - FREE :])
            else:
                body()

    nc.compile()
    return nc


def _make_runner(nc):
    """Persistent jitted shard_map over 8 cores (axon/PJRT path): the jit is
    built once, and the donated output buffers are created on-device so the
    1 GiB of zeros is never shipped over the transport."""
    import jax
    from jax.experimental.shard_map import shard_map
    from jax.sharding import Mesh, NamedSharding, PartitionSpec

    from concourse import bass2jax
    from concourse.bass2jax import _bass_exec_p, install_neuronx_cc_hook

    install_neuronx_cc_hook()

    partition_name = (
        nc.partition_id_tensor.name if nc.partition_id_tensor else None
    )
    in_names, out_names, out_avals = [], [], []
    for alloc in nc.m.functions[0].allocations:
        if not isinstance(alloc, mybir.MemoryLocationSet):
            continue
        name = alloc.memorylocations[0].name
        if alloc.kind == "ExternalInput":
            if name != partition_name:
                in_names.append(name)
        elif alloc.kind == "ExternalOutput":
            out_names.append(name)
            out_avals.append(
                jax.core.ShapedArray(tuple(alloc.tensor_shape),
                                     mybir.dt.np(alloc.dtype))
            )
    n_params = len(in_names)
    n_outs = len(out_avals)
    all_in_names = in_names + out_names
    if partition_name is not None:
        all_in_names = all_in_names + [partition_name]

    def _body(*args):
        operands = list(args)
        if partition_name is not None:
            operands.append(bass2jax.partition_id_tensor())
        return tuple(
            _bass_exec_p.bind(
                *operands,
                out_avals=tuple(out_avals),
                in_names=tuple(all_in_names),
                out_names=tuple(out_names),
                lowering_input_output_aliases=(),
                sim_require_finite=True,
                sim_require_nnan=True,
                nc=nc,
            )
        )

    devices = jax.devices()[:N_CORES]
    mesh = Mesh(np.asarray(devices), ("core",))
    in_specs = (PartitionSpec("core"),) * (n_params + n_outs)
    out_specs = (PartitionSpec("core"),) * n_outs
    donate = tuple(range(n_params, n_params + n_outs))
    sharded = jax.jit(
        shard_map(_body, mesh=mesh, in_specs=in_specs, out_specs=out_specs,
                  check_rep=False),
        donate_argnums=donate,
        keep_unused=True,
    )
    sh = NamedSharding(mesh, PartitionSpec("core"))
    zero_shapes = [
        ((N_CORES * a.shape[0], *a.shape[1:]), a.dtype) for a in out_avals
    ]
    dev_zeros = jax.jit(
        lambda: tuple(
            jax.numpy.zeros(s, d) for s, d in zero_shapes
        ),
        out_shardings=(sh,) * n_outs,
    )

    def run(in_maps):
        import jax.numpy as jnp  # noqa: F401

        concat_in = [
            np.concatenate([np.asarray(in_maps[c][nm]) for c in range(N_CORES)],
                           axis=0)
            for nm in in_names
        ]
        zeros = dev_zeros()
        outs = sharded(*concat_in, *zeros)
        outs = [np.asarray(o) for o in outs]
        return [
            {
                name: outs[i].reshape(N_CORES, *out_avals[i].shape)[c]
                for i, name in enumerate(out_names)
            }
            for c in range(N_CORES)
        ]

    return run


def kernel(z0: np.ndarray, kernel: np.ndarray, T) -> np.ndarray:
    global _NC_CACHE, _LAST_RESULT, _RUNNER
    assert int(T) == T_STEPS, f"kernel hardcodes T={T_STEPS}, got {T}"
    assert z0.shape == (B, D) and kernel.shape == (D, D)

    in_maps = [dict(m) for m in host_prep(z0, kernel)]

    if _NC_CACHE is None:
        _NC_CACHE = _build()

    from concourse.bass_utils import axon_active

    if axon_active() and not _PROFILE:
        if _RUNNER is None:
            _RUNNER = _make_runner(_NC_CACHE)
        results = _RUNNER(in_maps)
    else:
        res = run_bass_kernel_spmd(
            _NC_CACHE, in_maps, list(range(N_CORES)), trace=_PROFILE
        )
        _LAST_RESULT = res
        results = res.results

    out = np.empty((B, T_STEPS, D), np.float32)
    for m in range(N_CORES):
        out[m * BL : (m + 1) * BL] = results[m]["out"].reshape(BL, T_STEPS, D)
    return out


def host_prep(z0: np.ndarray, kmat: np.ndarray):
    """expm + powers in f64, per-core input maps."""
    k64 = _expm64(np.asarray(kmat, np.float64) * DT)
    pows = []
    p = np.eye(D)
    for _ in range(S):
        p = p @ k64
        pows.append(p.astype(np.float32))
    kcat = np.ascontiguousarray(np.concatenate(pows, axis=1))  # [D, S*D]
    ks = np.ascontiguousarray(pows[S - 1])  # [D, D]

    z0 = np.ascontiguousarray(z0, np.float32)
    in_maps = []
    for m in range(N_CORES):
        ztm = np.ascontiguousarray(z0[m * BL : (m + 1) * BL, :].T)  # [D, BL]
        in_maps.append({"zt": ztm, "kcat": kcat, "ks": ks})
    return in_maps
